# revision 1
# baseline (speedup 1.0000x reference)
import sys

sys.path.insert(0, "/opt/trn_rl_repo")

import numpy as np
import ml_dtypes

# ---- problem constants (hardcoded from the nn_LocalAggregator spec) ----
PC_MIN = np.array([-40.0, -40.0, -1.0], dtype=np.float32)
GRID = np.float32(0.4)
SCALE_MULT = np.float32(3.0)
N_PTS, N_GAUSS, N_CLS = 16384, 4096, 18
N_CORES = 8
NPC = N_PTS // N_CORES          # 2048 points per core
BLK = 512                       # point block (matmul free dim)
NBLK = NPC // BLK               # 4
P = 128                         # partitions / gaussians per tile
BIG = np.float64(1024.0)        # mask penalty (one violated axis is enough)
DUMMY_BIAS = -30000.0           # exp(-30000) == 0 exactly in fp32
KQ = 9                          # quadratic+linear monomial features
NSPLIT = [(0, 0), (0, 1), (1, 0), (1, 1), (0, 2), (2, 0)]  # bf16 split combos
KQR = KQ * len(NSPLIT)          # quad rows after splitting
KZ = 16                         # z voxel range
Y_CULL = True

BF16 = ml_dtypes.bfloat16

# module global for test harness introspection (exec time etc.)
LAST_RESULTS = None


def _split3(x):
    """float64 array -> 3 bf16 levels whose sum ~= x to ~24 bits."""
    a = x.astype(BF16)
    r = x - a.astype(np.float64)
    b = r.astype(BF16)
    r = r - b.astype(np.float64)
    c = r.astype(BF16)
    return a, b, c


def _prep(pts, means3D, opacities, semantics, scales, cov3D):
    """Host-side O(N+M) prep: sharding, features, coefficient tables."""
    p = np.asarray(pts[0], dtype=np.float32)          # [N,3]
    mu = np.asarray(means3D[0], dtype=np.float32)     # [M,3]
    opa = np.asarray(opacities[0], dtype=np.float32)  # [M]
    sem = np.asarray(semantics[0], dtype=np.float32)  # [M,C]
    sc = np.asarray(scales[0], dtype=np.float32)      # [M,3]
    cov = np.asarray(cov3D[0], dtype=np.float32)      # [M,3,3]

    # integer voxel coords / radii -- fp32 ops exactly as the reference
    p_int = ((p - PC_MIN) / GRID).astype(np.int32)
    m_int = ((mu - PC_MIN) / GRID).astype(np.int32)
    radii = np.ceil(sc.max(axis=-1) * SCALE_MULT / GRID).astype(np.int32)

    # symmetric precision entries, same picks as reference
    cxx = cov[:, 0, 0].astype(np.float64)
    cyy = cov[:, 1, 1].astype(np.float64)
    czz = cov[:, 2, 2].astype(np.float64)
    cxy = cov[:, 0, 1].astype(np.float64)
    cyz = cov[:, 1, 2].astype(np.float64)
    cxz = cov[:, 0, 2].astype(np.float64)
    with np.errstate(divide="ignore"):
        lnopa = np.log(opa.astype(np.float64))
    lnopa = np.maximum(lnopa, -20000.0)

    # ---- shard points: equal x-chunks, y-sorted inside each core ----
    order_x = np.argsort(p[:, 0], kind="stable")
    core_idx = []
    for c in range(N_CORES):
        idx = order_x[c * NPC:(c + 1) * NPC]
        idx = idx[np.argsort(p[idx, 1], kind="stable")]
        core_idx.append(idx)

    # ---- per-core gaussian subsets (x-reach cull), y-sorted ----
    core_gsel = []
    for c in range(N_CORES):
        vx = p_int[core_idx[c], 0]
        m = (m_int[:, 0] >= vx.min() - radii) & (m_int[:, 0] <= vx.max() + radii)
        gs = np.nonzero(m)[0]
        gs = gs[np.argsort(m_int[gs, 1], kind="stable")]
        core_gsel.append(gs)
    g_pad = P * int(np.ceil(max(len(g) for g in core_gsel) / P))
    n_gt = g_pad // P

    # ---- structural sizes shared across cores (SPMD) ----
    kx = 0
    ky = 0
    for c in range(N_CORES):
        vx = p_int[core_idx[c], 0]
        kx = max(kx, int(vx.max() - vx.min()) + 1)
        vy = p_int[core_idx[c], 1]
        for b in range(NBLK):
            vyb = vy[b * BLK:(b + 1) * BLK]
            ky = max(ky, int(vyb.max() - vyb.min()) + 1)
    ktot = KQR + kx + ky + KZ

    # ---- pair lists (g-tile x block), y-culled, padded across cores ----
    core_pairs = []   # per core: list of (b, t) with t = -1 for dummy
    counts = np.zeros((N_CORES, NBLK), dtype=np.int64)
    per_core_tb = []
    for c in range(N_CORES):
        gs = core_gsel[c]
        vy = p_int[core_idx[c], 1]
        tb = []
        for b in range(NBLK):
            vyb = vy[b * BLK:(b + 1) * BLK]
            ylo, yhi = int(vyb.min()), int(vyb.max())
            lst = []
            for t in range(n_gt):
                gg = gs[t * P:(t + 1) * P]
                if len(gg) == 0:
                    continue
                gl = (m_int[gg, 1] - radii[gg]).min()
                gh = (m_int[gg, 1] + radii[gg]).max()
                if (not Y_CULL) or (gl <= yhi and gh >= ylo):
                    lst.append(t)
            tb.append(lst)
            counts[c, b] = len(lst)
        per_core_tb.append(tb)
    npb = counts.max(axis=0)          # padded per-block pair counts
    npair = int(npb.sum())
    pair_block = []                    # baked structure, same for all cores
    for b in range(NBLK):
        pair_block += [b] * int(npb[b])

    # ---- per-core device arrays ----
    in_maps = []
    for c in range(N_CORES):
        idx = core_idx[c]
        gs = core_gsel[c]
        vx = p_int[idx, 0]
        vy = p_int[idx, 1]
        vz = p_int[idx, 2]
        vx_lo = int(vx.min())
        pc = p[idx].astype(np.float64)     # [NPC,3]

        # per-block centers
        centers = np.stack([pc[b * BLK:(b + 1) * BLK].mean(axis=0)
                            for b in range(NBLK)])   # [NBLK,3]
        ylos = [int(vy[b * BLK:(b + 1) * BLK].min()) for b in range(NBLK)]

        # ---- feature matrix FEAT [ktot, NPC] bf16 ----
        feat = np.zeros((ktot, NPC), dtype=BF16)
        for b in range(NBLK):
            cols = slice(b * BLK, (b + 1) * BLK)
            d = pc[cols] - centers[b]                  # [BLK,3] float64
            x, y, z = d[:, 0], d[:, 1], d[:, 2]
            q = np.stack([x * x, y * y, z * z, x * y, y * z, x * z, x, y, z])
            qs = _split3(q)                            # 3 x [KQ, BLK]
            for f in range(KQ):
                for k, (i, _) in enumerate(NSPLIT):
                    feat[f * len(NSPLIT) + k, cols] = qs[i][f]
            # one-hots
            rx = KQR + (vx[cols] - vx_lo)
            ryy = KQR + kx + (vy[cols] - ylos[b])
            rz = KQR + kx + ky + vz[cols]
            ar = np.arange(b * BLK, (b + 1) * BLK)
            feat[rx, ar] = BF16(1)
            feat[ryy, ar] = BF16(1)
            feat[rz, ar] = BF16(1)

        # ---- per-pair stationaries / biases / sem tiles ----
        stat = np.zeros((npair, ktot, P), dtype=BF16)
        bias = np.full((P, npair), DUMMY_BIAS, dtype=np.float32)
        semt = np.zeros((P, npair * N_CLS), dtype=np.float32)
        pi = 0
        for b in range(NBLK):
            lst = per_core_tb[c][b]
            for j in range(int(npb[b])):
                if j < len(lst):
                    t = lst[j]
                    gg = gs[t * P:(t + 1) * P]
                    ng = len(gg)
                    mup = mu[gg].astype(np.float64) - centers[b]  # [ng,3]
                    mx, my, mz = mup[:, 0], mup[:, 1], mup[:, 2]
                    gxx, gyy, gzz = cxx[gg], cyy[gg], czz[gg]
                    gxy, gyz, gxz = cxy[gg], cyz[gg], cxz[gg]
                    hx = gxx * mx + gxy * my + gxz * mz
                    hy = gxy * mx + gyy * my + gyz * mz
                    hz = gxz * mx + gyz * my + gzz * mz
                    gq = np.stack([-0.5 * gxx, -0.5 * gyy, -0.5 * gzz,
                                   -gxy, -gyz, -gxz, hx, hy, hz])  # [KQ,ng]
                    gsp = _split3(gq)
                    for f in range(KQ):
                        for k, (_, jj) in enumerate(NSPLIT):
                            stat[pi, f * len(NSPLIT) + k, :ng] = gsp[jj][f]
                    # interval tables (0 within reach, -BIG outside)
                    vv = np.arange(kx)[:, None] + vx_lo
                    out_x = np.abs(vv - m_int[gg, 0][None, :]) > radii[gg][None, :]
                    stat[pi, KQR:KQR + kx, :ng] = np.where(out_x, -BIG, 0.0).astype(BF16)
                    vv = np.arange(ky)[:, None] + ylos[b]
                    out_y = np.abs(vv - m_int[gg, 1][None, :]) > radii[gg][None, :]
                    stat[pi, KQR + kx:KQR + kx + ky, :ng] = np.where(out_y, -BIG, 0.0).astype(BF16)
                    vv = np.arange(KZ)[:, None]
                    out_z = np.abs(vv - m_int[gg, 2][None, :]) > radii[gg][None, :]
                    stat[pi, KQR + kx + ky:, :ng] = np.where(out_z, -BIG, 0.0).astype(BF16)
                    # bias: -0.5 mu'^T C mu' + ln(opa)
                    quad = (gxx * mx * mx + gyy * my * my + gzz * mz * mz
                            + 2 * gxy * mx * my + 2 * gyz * my * mz + 2 * gxz * mx * mz)
                    bias[:ng, pi] = (-0.5 * quad + lnopa[gg]).astype(np.float32)
                    semt[:ng, pi * N_CLS:(pi + 1) * N_CLS] = sem[gg]
                pi += 1

        # chunk layout: stat rows as [kchunks, 128, P] padded
        nchunks = int(np.ceil(ktot / P))
        kpad = nchunks * P
        featp = np.zeros((kpad, NPC), dtype=BF16)
        featp[:ktot] = feat
        statp = np.zeros((npair, kpad, P), dtype=BF16)
        statp[:, :ktot] = stat
        statt = statp.reshape(npair, nchunks, P, P)  # [pair, chunk, krow, g]
        statt = statt.transpose(2, 1, 0, 3).reshape(P, nchunks * npair * P)
        # rows = krow partition (128), cols = (chunk, pair, gauss)

        in_maps.append({
            "feat": featp.reshape(nchunks, P, NPC).transpose(1, 0, 2).reshape(P, nchunks * NPC),
            "stat": statt,
            "bias": bias,
            "semt": semt,
        })

    meta = dict(npair=npair, pair_block=pair_block, nchunks=nchunks,
                core_idx=core_idx, npb=npb, ktot=ktot)
    return in_maps, meta


def _build_nc(npair, pair_block, nchunks, ktot):
    import concourse.bass as bass  # noqa: F401
    import concourse.mybir as mybir
    import concourse.tile as tile
    from concourse import bacc

    f32 = mybir.dt.float32
    bf16 = mybir.dt.bfloat16

    nc = bacc.Bacc("TRN2", target_bir_lowering=False, debug=False,
                   num_devices=N_CORES)
    feat_d = nc.dram_tensor("feat", [P, nchunks * NPC], bf16, kind="ExternalInput")
    stat_d = nc.dram_tensor("stat", [P, nchunks * npair * P], bf16, kind="ExternalInput")
    bias_d = nc.dram_tensor("bias", [P, npair], f32, kind="ExternalInput")
    f32r = mybir.dt.float32r
    semt_d = nc.dram_tensor("semt", [P, npair * N_CLS], f32r, kind="ExternalInput")
    out_d = nc.dram_tensor("out", [N_CLS, NPC], f32, kind="ExternalOutput")

    # first/last pair index per block for psum accumulate flags
    first = {}
    last = {}
    for i, b in enumerate(pair_block):
        first.setdefault(b, i)
        last[b] = i

    with tile.TileContext(nc) as tc:
        with (
            tc.tile_pool(name="resident", bufs=1) as res_pool,
            tc.tile_pool(name="wpool", bufs=3) as w_pool,
            tc.tile_pool(name="pw", bufs=3, space="PSUM") as pw_pool,
            tc.tile_pool(name="lgp", bufs=1, space="PSUM") as lg_pool,
        ):
            feat_s = res_pool.tile([P, nchunks * NPC], bf16, name="feat_s")
            stat_s = res_pool.tile([P, nchunks * npair * P], bf16, name="stat_s")
            bias_s = res_pool.tile([P, npair], f32, name="bias_s")
            semt_s = res_pool.tile([P, npair * N_CLS], f32r, name="semt_s")
            out_s = res_pool.tile([N_CLS, NPC], f32, name="out_s")

            # stage inputs: few chunky DMAs, spread across issue queues, and
            # skip the all-zero padding rows of the last K chunk entirely
            nc.gpsimd.dma_start(out=bias_s[:], in_=bias_d[:])
            nc.gpsimd.dma_start(out=semt_s[:], in_=semt_d[:])
            krows = [min(P, ktot - ch * P) for ch in range(nchunks)]
            for ch in range(nchunks):
                nc.scalar.dma_start(
                    out=feat_s[:krows[ch], ch * NPC:(ch + 1) * NPC],
                    in_=feat_d[:krows[ch], ch * NPC:(ch + 1) * NPC])
            for ch in range(nchunks):
                ngrp = 3 if ch == 0 else 1
                bounds = [npair * g // ngrp for g in range(ngrp + 1)]
                for g in range(ngrp):
                    lo = (ch * npair + bounds[g]) * P
                    hi = (ch * npair + bounds[g + 1]) * P
                    nc.sync.dma_start(out=stat_s[:krows[ch], lo:hi],
                                      in_=stat_d[:krows[ch], lo:hi])

            lg = [lg_pool.tile([N_CLS, BLK], f32, name=f"lg{b}")
                  for b in range(NBLK)]

            for i, b in enumerate(pair_block):
                cols = slice(b * BLK, (b + 1) * BLK)
                pw = pw_pool.tile([P, BLK], f32, name="pw")
                for ch in range(nchunks):
                    kr = krows[ch]
                    lhs = stat_s[:kr, (ch * npair + i) * P:(ch * npair + i + 1) * P]
                    rhs = feat_s[:kr, ch * NPC + b * BLK: ch * NPC + (b + 1) * BLK]
                    nc.tensor.matmul(out=pw[:], lhsT=lhs, rhs=rhs,
                                     start=(ch == 0), stop=(ch == nchunks - 1))
                w = w_pool.tile([P, BLK], f32r, name="w")
                nc.scalar.activation(w[:], pw[:],
                                     mybir.ActivationFunctionType.Exp,
                                     bias=bias_s[:, i:i + 1])
                nc.tensor.matmul(out=lg[b][:],
                                 lhsT=semt_s[:, i * N_CLS:(i + 1) * N_CLS],
                                 rhs=w[:],
                                 start=(first[b] == i), stop=(last[b] == i))

            for b in range(NBLK):
                nc.vector.tensor_copy(out_s[:, b * BLK:(b + 1) * BLK], lg[b][:])
            nc.sync.dma_start(out=out_d[:], in_=out_s[:])

    nc.compile()
    return nc


def kernel(pts, means3D, opacities, semantics, scales, cov3D):
    global LAST_RESULTS
    from concourse.bass_utils import run_bass_kernel_spmd

    in_maps, meta = _prep(pts, means3D, opacities, semantics, scales, cov3D)
    nc = _build_nc(meta["npair"], meta["pair_block"], meta["nchunks"],
                   meta["ktot"])
    res = run_bass_kernel_spmd(nc, in_maps, core_ids=list(range(N_CORES)))
    LAST_RESULTS = res

    out = np.empty((N_PTS, N_CLS), dtype=np.float32)
    for c in range(N_CORES):
        out[meta["core_idx"][c]] = res.results[c]["out"].T
    return out



# revision 3
# speedup vs baseline: 1.2312x; 1.2312x over previous
import sys

sys.path.insert(0, "/opt/trn_rl_repo")

import numpy as np
import ml_dtypes

# ---- problem constants (hardcoded from the nn_LocalAggregator spec) ----
BF16 = ml_dtypes.bfloat16
PC_MIN = np.array([-40.0, -40.0, -1.0], dtype=np.float32)
GRID = np.float32(0.4)
SCALE_MULT = np.float32(3.0)
N_PTS, N_GAUSS, N_CLS = 16384, 4096, 18
N_CORES = 8
BLK = 512
P = 128
KQ = 9
COMBOS6 = [(0, 0), (0, 1), (1, 0), (1, 1), (0, 2), (2, 0)]
COMBOS3 = [(0, 0), (0, 1), (1, 0)]
BIGM = np.float64(1024.0)
DUMMY = np.float64(-30000.0)

LAST_RESULTS = None


def _split3(x):
    """float64 -> 3 bf16 levels whose sum ~= x to ~24 bits."""
    a = x.astype(BF16)
    r = x - a.astype(np.float64)
    b = r.astype(BF16)
    c = (r - b.astype(np.float64)).astype(BF16)
    return [a, b, c]


def _prep(pts, means3D, opacities, semantics, scales, cov3D):
    """Host-side O(N+M) prep: spatial blocks, features, coefficient tables.

    The input point cloud is block-sparse: 16 cells of ~10x10x1.6m. Points
    are split into x-columns at the 10m boundaries, y-sorted, and chopped
    into 512-point blocks (final block may overlap its neighbor; duplicate
    points compute identical logits so either copy is valid). Each block
    gathers its in-reach gaussians by exact 3-axis voxel-interval overlap.
    The gaussian exponent + cutoff mask + bias are evaluated as ONE
    [ktot<=128] x [512] matmul per (gaussian-tile, block) pair:
      rows = quadratic monomials (3-level bf16 split, 6 product combos)
           + x/y/z voxel one-hots whose per-gaussian interval tables also
             carry the bias (split 3 ways, exact for unmasked entries).
    """
    p = np.asarray(pts[0], dtype=np.float32)
    mu = np.asarray(means3D[0], dtype=np.float32)
    opa = np.asarray(opacities[0], dtype=np.float32)
    sem = np.asarray(semantics[0], dtype=np.float32)
    sc = np.asarray(scales[0], dtype=np.float32)
    cov = np.asarray(cov3D[0], dtype=np.float32)

    p_int = ((p - PC_MIN) / GRID).astype(np.int32)
    m_int = ((mu - PC_MIN) / GRID).astype(np.int32)
    radii = np.ceil(sc.max(axis=-1) * SCALE_MULT / GRID).astype(np.int32)

    c64 = cov.astype(np.float64)
    cxx, cyy, czz = c64[:, 0, 0], c64[:, 1, 1], c64[:, 2, 2]
    cxy, cyz, cxz = c64[:, 0, 1], c64[:, 1, 2], c64[:, 0, 2]
    with np.errstate(divide="ignore"):
        lnopa = np.maximum(np.log(opa.astype(np.float64)), -20000.0)

    col = np.clip(((p[:, 0] + 40.0) // 10.0).astype(np.int64), 0, 7)
    blocks = []
    for o in range(8):
        idx = np.nonzero(col == o)[0]
        idx = idx[np.argsort(p[idx, 1], kind="stable")]
        n = len(idx)
        nb = max(1, -(-n // BLK))
        starts = [i * BLK for i in range(nb - 1)] + [n - BLK]
        for s in starts:
            blocks.append(idx[s:s + BLK])

    binfo = []
    for idx in blocks:
        vx, vy, vz = p_int[idx, 0], p_int[idx, 1], p_int[idx, 2]
        lo = np.array([vx.min(), vy.min(), vz.min()])
        hi = np.array([vx.max(), vy.max(), vz.max()])
        m = np.ones(N_GAUSS, bool)
        for a in range(3):
            m &= (m_int[:, a] + radii >= lo[a]) & (m_int[:, a] - radii <= hi[a])
        g = np.nonzero(m)[0]
        binfo.append(dict(idx=idx, g=g, lo=lo, hi=hi, ntile=-(-len(g) // P)))

    kx = max(int(b["hi"][0] - b["lo"][0]) for b in binfo) + 1
    ky = max(int(b["hi"][1] - b["lo"][1]) for b in binfo) + 1
    kz = max(int(b["hi"][2] - b["lo"][2]) for b in binfo) + 1
    combos = COMBOS6 if 54 + kx + ky + kz <= 128 else COMBOS3
    kqr = KQ * len(combos)
    ktot = kqr + kx + ky + kz
    assert ktot <= 128, f"ktot={ktot} exceeds one chunk"
    xo, yo, zo = kqr, kqr + kx, kqr + kx + ky

    # pack blocks into cores (serpentine by tile count), sort slots desc
    order = sorted(range(len(blocks)), key=lambda i: -binfo[i]["ntile"])
    coreblocks = [[] for _ in range(N_CORES)]
    for k, bi in enumerate(order):
        rnd, pos = divmod(k, N_CORES)
        c = pos if rnd % 2 == 0 else N_CORES - 1 - pos
        coreblocks[c].append(bi)
    npb = max(len(cb) for cb in coreblocks)
    for cb in coreblocks:
        cb.sort(key=lambda i: -binfo[i]["ntile"])
    slot_pairs = [max(binfo[cb[s]]["ntile"] if s < len(cb) else 0
                      for cb in coreblocks) for s in range(npb)]
    npair = int(sum(slot_pairs))
    pair_block = []
    for s, np_ in enumerate(slot_pairs):
        pair_block += [s] * np_

    in_maps = []
    core_ids_pts = []
    for c in range(N_CORES):
        feat = np.zeros((ktot, npb * BLK), dtype=BF16)
        stat = np.zeros((ktot, npair * P), dtype=BF16)
        semt = np.zeros((P, npair * N_CLS), dtype=np.float32)
        ids = np.full(npb * BLK, -1, dtype=np.int64)
        pi0 = 0
        for s in range(npb):
            npairs_s = slot_pairs[s]
            if s >= len(coreblocks[c]):
                for pi in range(pi0, pi0 + npairs_s):
                    stat[zo:zo + kz, pi * P:(pi + 1) * P] = BF16(DUMMY)
                pi0 += npairs_s
                continue
            b = binfo[coreblocks[c][s]]
            idx, g, lo = b["idx"], b["g"], b["lo"]
            ids[s * BLK:(s + 1) * BLK] = idx
            pc = p[idx].astype(np.float64)
            center = pc.mean(axis=0)
            d = pc - center
            x, y, z = d[:, 0], d[:, 1], d[:, 2]
            q = np.stack([x * x, y * y, z * z, x * y, y * z, x * z, x, y, z])
            qs = _split3(q)
            cols = slice(s * BLK, (s + 1) * BLK)
            for f in range(KQ):
                for k, (i, _) in enumerate(combos):
                    feat[f * len(combos) + k, cols] = qs[i][f]
            ar = np.arange(s * BLK, (s + 1) * BLK)
            feat[xo + (p_int[idx, 0] - lo[0]), ar] = BF16(1)
            feat[yo + (p_int[idx, 1] - lo[1]), ar] = BF16(1)
            feat[zo + (p_int[idx, 2] - lo[2]), ar] = BF16(1)

            for t in range(npairs_s):
                pi = pi0 + t
                gg = g[t * P:(t + 1) * P]
                ng = len(gg)
                if ng == 0:
                    stat[zo:zo + kz, pi * P:(pi + 1) * P] = BF16(DUMMY)
                    continue
                gcols = slice(pi * P, pi * P + ng)
                mup = mu[gg].astype(np.float64) - center
                mx, my, mz = mup[:, 0], mup[:, 1], mup[:, 2]
                gxx, gyy, gzz = cxx[gg], cyy[gg], czz[gg]
                gxy, gyz, gxz = cxy[gg], cyz[gg], cxz[gg]
                hx = gxx * mx + gxy * my + gxz * mz
                hy = gxy * mx + gyy * my + gyz * mz
                hz = gxz * mx + gyz * my + gzz * mz
                gq = np.stack([-0.5 * gxx, -0.5 * gyy, -0.5 * gzz,
                               -gxy, -gyz, -gxz, hx, hy, hz])
                gsp = _split3(gq)
                for f in range(KQ):
                    for k, (_, j) in enumerate(combos):
                        stat[f * len(combos) + k, gcols] = gsp[j][f]
                quad = (gxx * mx * mx + gyy * my * my + gzz * mz * mz
                        + 2 * gxy * mx * my + 2 * gyz * my * mz
                        + 2 * gxz * mx * mz)
                bias = -0.5 * quad + lnopa[gg]
                bh = bias.astype(BF16).astype(np.float64)
                bm = (bias - bh).astype(BF16).astype(np.float64)
                bl = (bias - bh - bm).astype(BF16)
                vv = np.arange(kx)[:, None] + lo[0]
                stat[xo:xo + kx, gcols] = (np.where(
                    np.abs(vv - m_int[gg, 0]) > radii[gg], -BIGM, 0.0)
                    + bl.astype(np.float64)).astype(BF16)
                vv = np.arange(ky)[:, None] + lo[1]
                stat[yo:yo + ky, gcols] = (np.where(
                    np.abs(vv - m_int[gg, 1]) > radii[gg], -BIGM, 0.0)
                    + bm).astype(BF16)
                vv = np.arange(kz)[:, None] + lo[2]
                stat[zo:zo + kz, gcols] = (np.where(
                    np.abs(vv - m_int[gg, 2]) > radii[gg], -BIGM, 0.0)
                    + bh).astype(BF16)
                if ng < P:
                    stat[zo:zo + kz, pi * P + ng:(pi + 1) * P] = BF16(DUMMY)
                semt[:ng, pi * N_CLS:(pi + 1) * N_CLS] = sem[gg]
            pi0 += npairs_s

        in_maps.append({"feat": feat, "stat": stat, "semt": semt})
        core_ids_pts.append(ids)

    first = {}
    last = {}
    for i, b in enumerate(pair_block):
        first.setdefault(b, i)
        last[b] = i
    meta = dict(ktot=ktot, npb=npb, npair=npair, pair_block=pair_block,
                first=first, last=last, ids=core_ids_pts,
                split_pairs=slot_pairs[0] + (slot_pairs[1] if npb > 1 else 0),
                split_blocks=min(2, npb))
    return in_maps, meta


def _build_nc(ktot, npb, npair, pair_block, first, last, split_pairs,
              split_blocks):
    import concourse.bass as bass  # noqa: F401
    import concourse.mybir as mybir
    import concourse.tile as tile
    from concourse import bacc

    f32 = mybir.dt.float32
    f32r = mybir.dt.float32r
    bf16 = mybir.dt.bfloat16

    nc = bacc.Bacc("TRN2", target_bir_lowering=False, debug=False,
                   num_devices=N_CORES)
    feat_d = nc.dram_tensor("feat", [ktot, npb * BLK], bf16,
                            kind="ExternalInput")
    stat_d = nc.dram_tensor("stat", [ktot, npair * P], bf16,
                            kind="ExternalInput")
    semt_d = nc.dram_tensor("semt", [P, npair * N_CLS], f32r,
                            kind="ExternalInput")
    out_d = nc.dram_tensor("out", [N_CLS, npb * BLK], f32,
                           kind="ExternalOutput")

    with tile.TileContext(nc) as tc:
        with (
            tc.tile_pool(name="resident", bufs=1) as res_pool,
            tc.tile_pool(name="wpool", bufs=3) as w_pool,
            tc.tile_pool(name="pw", bufs=3, space="PSUM") as pw_pool,
            tc.tile_pool(name="lgp", bufs=2, space="PSUM") as lg_pool,
        ):
            feat_s = res_pool.tile([ktot, npb * BLK], bf16, name="feat_s")
            stat_s = res_pool.tile([ktot, npair * P], bf16, name="stat_s")
            semt_s = res_pool.tile([P, npair * N_CLS], f32r, name="semt_s")
            out_s = res_pool.tile([N_CLS, npb * BLK], f32, name="out_s")

            # stage inputs: first slices needed by early pairs, then the rest
            sp = split_pairs
            sb = split_blocks
            nc.sync.dma_start(out=stat_s[:, :sp * P], in_=stat_d[:, :sp * P])
            nc.scalar.dma_start(out=feat_s[:, :sb * BLK],
                                in_=feat_d[:, :sb * BLK])
            nc.gpsimd.dma_start(out=semt_s[:], in_=semt_d[:])
            if sp < npair:
                nc.sync.dma_start(out=stat_s[:, sp * P:],
                                  in_=stat_d[:, sp * P:])
            if sb < npb:
                nc.scalar.dma_start(out=feat_s[:, sb * BLK:],
                                    in_=feat_d[:, sb * BLK:])

            lg = {}
            for i0 in range(0, npair, 2):
                n2 = min(2, npair - i0)
                pw = pw_pool.tile([P, n2 * BLK], f32, name="pw")
                for j in range(n2):
                    i = i0 + j
                    b = pair_block[i]
                    nc.tensor.matmul(
                        out=pw[:, j * BLK:(j + 1) * BLK],
                        lhsT=stat_s[:, i * P:(i + 1) * P],
                        rhs=feat_s[:, b * BLK:(b + 1) * BLK],
                        start=True, stop=True)
                w = w_pool.tile([P, n2 * BLK], f32r, name="w")
                nc.scalar.activation(w[:], pw[:],
                                     mybir.ActivationFunctionType.Exp)
                for j in range(n2):
                    i = i0 + j
                    b = pair_block[i]
                    if first[b] == i:
                        lg[b] = lg_pool.tile([N_CLS, BLK], f32, name="lg")
                    nc.tensor.matmul(
                        out=lg[b][:],
                        lhsT=semt_s[:, i * N_CLS:(i + 1) * N_CLS],
                        rhs=w[:, j * BLK:(j + 1) * BLK],
                        start=(first[b] == i), stop=(last[b] == i))
                    if last[b] == i:
                        nc.vector.tensor_copy(
                            out_s[:, b * BLK:(b + 1) * BLK], lg[b][:])
                        nc.gpsimd.dma_start(
                            out=out_d[:, b * BLK:(b + 1) * BLK],
                            in_=out_s[:, b * BLK:(b + 1) * BLK])

    nc.compile()
    return nc


def kernel(pts, means3D, opacities, semantics, scales, cov3D):
    global LAST_RESULTS
    from concourse.bass_utils import run_bass_kernel_spmd

    in_maps, meta = _prep(pts, means3D, opacities, semantics, scales, cov3D)
    nc = _build_nc(meta["ktot"], meta["npb"], meta["npair"],
                   meta["pair_block"], meta["first"], meta["last"],
                   meta["split_pairs"], meta["split_blocks"])
    res = run_bass_kernel_spmd(nc, in_maps, core_ids=list(range(N_CORES)))
    LAST_RESULTS = res

    out = np.zeros((N_PTS, N_CLS), dtype=np.float32)
    for c in range(N_CORES):
        ids = meta["ids"][c]
        ok = ids >= 0
        out[ids[ok]] = res.results[c]["out"].T[ok]
    return out


# revision 4
# speedup vs baseline: 1.5715x; 1.2764x over previous
import sys

sys.path.insert(0, "/opt/trn_rl_repo")

import numpy as np
import ml_dtypes

# ---- problem constants (hardcoded from the nn_LocalAggregator spec) ----
BF16 = ml_dtypes.bfloat16
PC_MIN = np.array([-40.0, -40.0, -1.0], dtype=np.float32)
GRID = np.float32(0.4)
SCALE_MULT = np.float32(3.0)
N_PTS, N_GAUSS, N_CLS = 16384, 4096, 18
N_CORES = 8
BLK = 512
P = 128
KQ = 9
COMBOS = [(0, 0), (0, 1), (1, 0)]   # 2-level bf16 split products
BIGM = np.float64(1024.0)
DUMMY = np.float64(-30000.0)

LAST_RESULTS = None


def _split2(x):
    """float64 -> 2 bf16 levels whose sum ~= x to ~16 bits."""
    a = x.astype(BF16)
    b = (x - a.astype(np.float64)).astype(BF16)
    return [a, b]


def _prep(pts, means3D, opacities, semantics, scales, cov3D):
    """Host-side O(N+M) prep: spatial blocks, features, coefficient tables.

    The input point cloud is block-sparse: 16 cells of ~10x10x1.6m. Points
    are split into x-columns at the 10m boundaries, y-sorted, and chopped
    into 512-point blocks (final block may overlap its neighbor; duplicate
    points compute identical logits so either copy is valid). Each block
    gathers its in-reach gaussians by exact 3-axis voxel-interval overlap.
    The gaussian exponent + cutoff mask + bias are evaluated as ONE
    [ktot<=128] x [512] matmul per (gaussian-tile, block) pair:
      rows = quadratic monomials (2-level bf16 split, 3 product combos)
           + x/y/z voxel one-hots whose per-gaussian interval tables also
             carry the bias (split 3 ways, exact for unmasked entries).
    """
    p = np.asarray(pts[0], dtype=np.float32)
    mu = np.asarray(means3D[0], dtype=np.float32)
    opa = np.asarray(opacities[0], dtype=np.float32)
    sem = np.asarray(semantics[0], dtype=np.float32)
    sc = np.asarray(scales[0], dtype=np.float32)
    cov = np.asarray(cov3D[0], dtype=np.float32)

    p_int = ((p - PC_MIN) / GRID).astype(np.int32)
    m_int = ((mu - PC_MIN) / GRID).astype(np.int32)
    radii = np.ceil(sc.max(axis=-1) * SCALE_MULT / GRID).astype(np.int32)

    c64 = cov.astype(np.float64)
    cxx, cyy, czz = c64[:, 0, 0], c64[:, 1, 1], c64[:, 2, 2]
    cxy, cyz, cxz = c64[:, 0, 1], c64[:, 1, 2], c64[:, 0, 2]
    with np.errstate(divide="ignore"):
        lnopa = np.maximum(np.log(opa.astype(np.float64)), -20000.0)

    col = np.clip(((p[:, 0] + 40.0) // 10.0).astype(np.int64), 0, 7)
    blocks = []
    for o in range(8):
        idx = np.nonzero(col == o)[0]
        idx = idx[np.argsort(p[idx, 1], kind="stable")]
        n = len(idx)
        nb = max(1, -(-n // BLK))
        starts = [i * BLK for i in range(nb - 1)] + [n - BLK]
        for s in starts:
            blocks.append(idx[s:s + BLK])

    binfo = []
    for idx in blocks:
        vx, vy, vz = p_int[idx, 0], p_int[idx, 1], p_int[idx, 2]
        lo = np.array([vx.min(), vy.min(), vz.min()])
        hi = np.array([vx.max(), vy.max(), vz.max()])
        m = np.ones(N_GAUSS, bool)
        for a in range(3):
            m &= (m_int[:, a] + radii >= lo[a]) & (m_int[:, a] - radii <= hi[a])
        g = np.nonzero(m)[0]
        binfo.append(dict(idx=idx, g=g, lo=lo, hi=hi, ntile=-(-len(g) // P)))

    kx = max(int(b["hi"][0] - b["lo"][0]) for b in binfo) + 1
    ky = max(int(b["hi"][1] - b["lo"][1]) for b in binfo) + 1
    kz = max(int(b["hi"][2] - b["lo"][2]) for b in binfo) + 1
    kqr = KQ * len(COMBOS)
    ktot = kqr + kx + ky + kz
    assert ktot <= 128, f"ktot={ktot} exceeds one chunk"
    xo, yo, zo = kqr, kqr + kx, kqr + kx + ky

    # pack blocks into cores minimizing per-slot padded pair counts
    tiles = [b["ntile"] for b in binfo]
    nbl = len(blocks)
    npb = -(-nbl // N_CORES)

    def pack(order):
        cb = [[] for _ in range(N_CORES)]
        loads = [0] * N_CORES
        for bi in order:
            cands = sorted(range(N_CORES), key=lambda c: (loads[c], len(cb[c])))
            c = next(c for c in cands if len(cb[c]) < npb)
            cb[c].append(bi)
            loads[c] += tiles[bi]
        for l in cb:
            l.sort(key=lambda i: -tiles[i])
        sp = [max(tiles[l[s]] if s < len(l) else 0 for l in cb)
              for s in range(npb)]
        return cb, sp, sum(sp)

    best = pack(sorted(range(nbl), key=lambda i: -tiles[i]))
    rng = np.random.default_rng(0)
    for _ in range(64):
        cand = pack(list(rng.permutation(nbl)))
        if cand[2] < best[2]:
            best = cand
    coreblocks, slot_pairs, npair = best
    npair = int(npair)
    pair_block = []
    for s, np_ in enumerate(slot_pairs):
        pair_block += [s] * np_

    in_maps = []
    core_ids_pts = []
    for c in range(N_CORES):
        feat = np.zeros((ktot, npb * BLK), dtype=BF16)
        stat = np.zeros((ktot, npair * P), dtype=BF16)
        semt = np.zeros((P, npair * N_CLS), dtype=BF16)
        ids = np.full(npb * BLK, -1, dtype=np.int64)
        pi0 = 0
        for s in range(npb):
            npairs_s = slot_pairs[s]
            if s >= len(coreblocks[c]):
                for pi in range(pi0, pi0 + npairs_s):
                    stat[zo:zo + kz, pi * P:(pi + 1) * P] = BF16(DUMMY)
                pi0 += npairs_s
                continue
            b = binfo[coreblocks[c][s]]
            idx, g, lo = b["idx"], b["g"], b["lo"]
            ids[s * BLK:(s + 1) * BLK] = idx
            pc = p[idx].astype(np.float64)
            center = pc.mean(axis=0)
            d = pc - center
            x, y, z = d[:, 0], d[:, 1], d[:, 2]
            q = np.stack([x * x, y * y, z * z, x * y, y * z, x * z, x, y, z])
            qs = _split2(q)
            cols = slice(s * BLK, (s + 1) * BLK)
            for f in range(KQ):
                for k, (i, _) in enumerate(COMBOS):
                    feat[f * len(COMBOS) + k, cols] = qs[i][f]
            ar = np.arange(s * BLK, (s + 1) * BLK)
            feat[xo + (p_int[idx, 0] - lo[0]), ar] = BF16(1)
            feat[yo + (p_int[idx, 1] - lo[1]), ar] = BF16(1)
            feat[zo + (p_int[idx, 2] - lo[2]), ar] = BF16(1)

            for t in range(npairs_s):
                pi = pi0 + t
                gg = g[t * P:(t + 1) * P]
                ng = len(gg)
                if ng == 0:
                    stat[zo:zo + kz, pi * P:(pi + 1) * P] = BF16(DUMMY)
                    continue
                gcols = slice(pi * P, pi * P + ng)
                mup = mu[gg].astype(np.float64) - center
                mx, my, mz = mup[:, 0], mup[:, 1], mup[:, 2]
                gxx, gyy, gzz = cxx[gg], cyy[gg], czz[gg]
                gxy, gyz, gxz = cxy[gg], cyz[gg], cxz[gg]
                hx = gxx * mx + gxy * my + gxz * mz
                hy = gxy * mx + gyy * my + gyz * mz
                hz = gxz * mx + gyz * my + gzz * mz
                gq = np.stack([-0.5 * gxx, -0.5 * gyy, -0.5 * gzz,
                               -gxy, -gyz, -gxz, hx, hy, hz])
                gsp = _split2(gq)
                for f in range(KQ):
                    for k, (_, j) in enumerate(COMBOS):
                        stat[f * len(COMBOS) + k, gcols] = gsp[j][f]
                quad = (gxx * mx * mx + gyy * my * my + gzz * mz * mz
                        + 2 * gxy * mx * my + 2 * gyz * my * mz
                        + 2 * gxz * mx * mz)
                bias = -0.5 * quad + lnopa[gg]
                bh = bias.astype(BF16).astype(np.float64)
                bm = (bias - bh).astype(BF16).astype(np.float64)
                bl = (bias - bh - bm).astype(BF16)
                vv = np.arange(kx)[:, None] + lo[0]
                stat[xo:xo + kx, gcols] = (np.where(
                    np.abs(vv - m_int[gg, 0]) > radii[gg], -BIGM, 0.0)
                    + bl.astype(np.float64)).astype(BF16)
                vv = np.arange(ky)[:, None] + lo[1]
                stat[yo:yo + ky, gcols] = (np.where(
                    np.abs(vv - m_int[gg, 1]) > radii[gg], -BIGM, 0.0)
                    + bm).astype(BF16)
                vv = np.arange(kz)[:, None] + lo[2]
                stat[zo:zo + kz, gcols] = (np.where(
                    np.abs(vv - m_int[gg, 2]) > radii[gg], -BIGM, 0.0)
                    + bh).astype(BF16)
                if ng < P:
                    stat[zo:zo + kz, pi * P + ng:(pi + 1) * P] = BF16(DUMMY)
                semt[:ng, pi * N_CLS:(pi + 1) * N_CLS] = sem[gg].astype(BF16)
            pi0 += npairs_s

        in_maps.append({"feat": feat, "stat": stat, "semt": semt})
        core_ids_pts.append(ids)

    first = {}
    last = {}
    for i, b in enumerate(pair_block):
        first.setdefault(b, i)
        last[b] = i
    meta = dict(ktot=ktot, npb=npb, npair=npair, pair_block=pair_block,
                first=first, last=last, ids=core_ids_pts,
                slot_pairs=slot_pairs)
    return in_maps, meta


def _build_nc(ktot, npb, npair, pair_block, first, last, slot_pairs):
    import concourse.bass as bass  # noqa: F401
    import concourse.mybir as mybir
    import concourse.tile as tile
    from concourse import bacc

    f32 = mybir.dt.float32
    f16 = mybir.dt.float16
    bf16 = mybir.dt.bfloat16

    nc = bacc.Bacc("TRN2", target_bir_lowering=False, debug=False,
                   num_devices=N_CORES)
    feat_d = nc.dram_tensor("feat", [ktot, npb * BLK], bf16,
                            kind="ExternalInput")
    stat_d = nc.dram_tensor("stat", [ktot, npair * P], bf16,
                            kind="ExternalInput")
    semt_d = nc.dram_tensor("semt", [P, npair * N_CLS], bf16,
                            kind="ExternalInput")
    out_d = nc.dram_tensor("out", [N_CLS, npb * BLK], f16,
                           kind="ExternalOutput")

    # DMA split points (in pairs / blocks) chosen so the first compute can
    # start early while later chunks stream behind it.
    spA = min(slot_pairs[0], npair)          # pairs of slot 0
    sbA = 2                                   # feat blocks 0-1
    # out DMA split: blocks 0..npb-3 early, rest at end
    out_split = max(1, npb - 2)

    with tile.TileContext(nc) as tc:
        with (
            tc.tile_pool(name="resident", bufs=1) as res_pool,
            tc.tile_pool(name="wpool", bufs=3) as w_pool,
            tc.tile_pool(name="pw", bufs=3, space="PSUM") as pw_pool,
            tc.tile_pool(name="lgp", bufs=2, space="PSUM") as lg_pool,
        ):
            feat_s = res_pool.tile([ktot, npb * BLK], bf16, name="feat_s")
            stat_s = res_pool.tile([ktot, npair * P], bf16, name="stat_s")
            semt_s = res_pool.tile([P, npair * N_CLS], bf16, name="semt_s")
            out_s = res_pool.tile([N_CLS, npb * BLK], f16, name="out_s")

            # stage inputs in need-order across the three DMA queues
            nc.sync.dma_start(out=stat_s[:, :spA * P],
                              in_=stat_d[:, :spA * P])
            nc.scalar.dma_start(out=feat_s[:, :sbA * BLK],
                                in_=feat_d[:, :sbA * BLK])
            nc.gpsimd.dma_start(out=semt_s[:, :spA * N_CLS],
                                in_=semt_d[:, :spA * N_CLS])
            nc.sync.dma_start(out=stat_s[:, spA * P:],
                              in_=stat_d[:, spA * P:])
            nc.scalar.dma_start(out=feat_s[:, sbA * BLK:],
                                in_=feat_d[:, sbA * BLK:])
            nc.gpsimd.dma_start(out=semt_s[:, spA * N_CLS:],
                                in_=semt_d[:, spA * N_CLS:])

            lg = {}
            done_blocks = []
            for i0 in range(0, npair, 2):
                n2 = min(2, npair - i0)
                pw = pw_pool.tile([P, n2 * BLK], f32, name="pw")
                for j in range(n2):
                    i = i0 + j
                    b = pair_block[i]
                    nc.tensor.matmul(
                        out=pw[:, j * BLK:(j + 1) * BLK],
                        lhsT=stat_s[:, i * P:(i + 1) * P],
                        rhs=feat_s[:, b * BLK:(b + 1) * BLK],
                        start=True, stop=True)
                w = w_pool.tile([P, n2 * BLK], bf16, name="w")
                nc.scalar.activation(w[:], pw[:],
                                     mybir.ActivationFunctionType.Exp)
                for j in range(n2):
                    i = i0 + j
                    b = pair_block[i]
                    if first[b] == i:
                        lg[b] = lg_pool.tile([N_CLS, BLK], f32, name="lg")
                    nc.tensor.matmul(
                        out=lg[b][:],
                        lhsT=semt_s[:, i * N_CLS:(i + 1) * N_CLS],
                        rhs=w[:, j * BLK:(j + 1) * BLK],
                        start=(first[b] == i), stop=(last[b] == i))
                    if last[b] == i:
                        nc.vector.tensor_copy(
                            out_s[:, b * BLK:(b + 1) * BLK], lg[b][:])
                        done_blocks.append(b)
                        if len(done_blocks) == out_split:
                            nc.sync.dma_start(
                                out=out_d[:, :out_split * BLK],
                                in_=out_s[:, :out_split * BLK])
            nc.sync.dma_start(out=out_d[:, out_split * BLK:],
                              in_=out_s[:, out_split * BLK:])

    nc.compile()
    return nc


def kernel(pts, means3D, opacities, semantics, scales, cov3D):
    global LAST_RESULTS
    from concourse.bass_utils import run_bass_kernel_spmd

    in_maps, meta = _prep(pts, means3D, opacities, semantics, scales, cov3D)
    nc = _build_nc(meta["ktot"], meta["npb"], meta["npair"],
                   meta["pair_block"], meta["first"], meta["last"],
                   meta["slot_pairs"])
    res = run_bass_kernel_spmd(nc, in_maps, core_ids=list(range(N_CORES)))
    LAST_RESULTS = res

    out = np.zeros((N_PTS, N_CLS), dtype=np.float32)
    for c in range(N_CORES):
        ids = meta["ids"][c]
        ok = ids >= 0
        out[ids[ok]] = res.results[c]["out"].astype(np.float32).T[ok]
    return out
